# revision 5
# baseline (speedup 1.0000x reference)
"""AdaConv2d (per-pixel 3x3 dynamic conv) on 8 TRN2 NeuronCores.

out[b,c,h,w] = sum_t x_pad[b,c,h+dh(t),w+dw(t)] * dk[b,c,t,h,w]

Sharding: pure data parallel over batch (B=8 -> one batch element per core).

Per-core layout: partition p = 2c+s (c = channel 0..63, s = H-half 0..1); each
partition holds its unpadded half-plane of x ([64 rows x 128 cols], row f =
x[c, 64s+f]) resident in SBUF while dynamic_kernel streams through in
(row-block, tap) tiles. Tap shifts are free-dim window reads; W-boundary
columns are zeroed in the product tile; H-boundary rows use parity-masked
halo row tiles (the mid-boundary halo lives in the neighbor partition, so it
is DMA'd separately and masked by a per-partition parity vector).

Engines: DVE computes the per-tap products (f32 in, bf16 out); the 9-tap sum
is accumulated on the TensorEngine via identity-matmul into PSUM (f32); ACT
drains PSUM to bf16 SBUF staging; output is stored bf16 and upcast on host.
Memory-bound: ~44 MB/core HBM traffic at ~360-400 GB/s/core.
"""

import numpy as np

from concourse import bacc, bass, tile
from concourse import mybir
from concourse.ap import AP
from concourse.bass_utils import run_bass_kernel_spmd
from concourse.masks import make_identity

F32 = mybir.dt.float32
BF16 = mybir.dt.bfloat16

B, C, H, W = 8, 64, 128, 128
K = 3
NTAP = K * K
NCORES = 8

HALF = H // 2           # 64 rows per half-plane
NBLK = 4                # row-blocks per half
RB = HALF // NBLK       # 16 rows per block
XROWS = HALF + 2        # 66 padded rows per partition
XCOLS = W + 2           # 130 padded cols

_CACHED_NC = None


def _emit(tc, nc, x_ap, dk_ap, pm_dram, out_ap):
    ctx_pools = []

    def pool(name, bufs, space=bass.MemorySpace.SBUF):
        p = tc.tile_pool(name=name, bufs=bufs, space=space)
        ctx_pools.append(p)
        return p.__enter__()

    try:
        const_pool = pool("const", 1)
        x_pool = pool("xp", 1)
        dk_pool = pool("dk", 6)
        tmp_pool = pool("tmp", 4)
        out_pool = pool("osb", 3)
        psum_pool = pool("ps", 8, space=bass.MemorySpace.PSUM)

        identity = const_pool.tile([128, 128], BF16, name="identity")
        make_identity(nc, identity)

        # Partition p = 2c+s (c = channel, s = H-half). x_tile row f holds
        # x[c, 64s+f] (no padding); every DMA spans all 128 partitions in ONE
        # dma_start with outer source dim 64 (c) and contiguous runs, else
        # engine/port splitting craters bandwidth. W-boundary tap columns are
        # memset to zero in tmp; H-boundary tap rows (top of block 0 /
        # bottom of block 1) come from parity-masked halo row tiles.
        pm_ap = const_pool.tile([128, 2], F32, name="pm_ap")
        nc.scalar.dma_start(out=pm_ap[:], in_=pm_dram)
        # halo_bot: even p (s=0) need x row 64, odd don't-care (row 65) -> *even
        # halo_top: odd p (s=1) need x row 63, even don't-care (row 62) -> *odd
        halo_raw = x_pool.tile([128, 2, W], F32, name="halo_raw")
        nc.scalar.dma_start(
            out=halo_raw[:, 0:1, :],
            in_=AP(x_ap.tensor, HALF * W, [[H * W, C], [W, 2], [1, W]]),
        )
        nc.scalar.dma_start(
            out=halo_raw[:, 1:2, :],
            in_=AP(x_ap.tensor, (HALF - 2) * W, [[H * W, C], [W, 2], [1, W]]),
        )
        # x is loaded on the SAME sync ring as the dk stream, in per-block
        # just-in-time chunks. One FIFO ring means block0's 18 x rows land
        # ~2us after kernel start, so the DVE muls begin immediately and the
        # dk stream never stalls on bufs backpressure waiting for x (the old
        # two-chunk x load on the scalar ring fair-shared the DMA engines
        # with dk and didn't complete until ~30us, idling everything).
        x_tile = x_pool.tile([128, HALF, W], F32, name="x_tile")

        def load_x_rows(lo, hi):
            nc.sync.dma_start(
                out=x_tile[:, lo:hi, :],
                in_=AP(
                    x_ap.tensor,
                    lo * W,
                    [[H * W, C], [HALF * W, 2], [W, hi - lo], [1, W]],
                ),
            )

        halo = x_pool.tile([128, 2, W], F32, name="halo")
        nc.gpsimd.tensor_scalar_mul(halo[:, 0:1, :], halo_raw[:, 0:1, :], pm_ap[:, 0:1])
        nc.gpsimd.tensor_scalar_mul(halo[:, 1:2, :], halo_raw[:, 1:2, :], pm_ap[:, 1:2])

        blocks = [16, 32, 16]  # >=8KB/lane packets for every dk load
        assert sum(blocks) == HALF
        nblocks = len(blocks)
        # x chunk boundaries: chunk b covers rows up to blocks[b] end + 2
        # (block b reads x rows r0-1 .. r0+rb, halo rows aside)
        xcuts = [18, 50, HALF]
        xdone = 0
        load_x_rows(0, xcuts[0])
        xdone = xcuts[0]
        # Per-block tap order: halo-edge taps (gpsimd mul, ~2us) must not be
        # the accumulation-closing tap — b0 runs its dh=-1 taps late (halo
        # mask prep has warmed up by then) but not last; the final block runs
        # its dh=+1 taps first so its stop-tap is a cheap vector-only mul.
        orders = [list(range(NTAP))] * nblocks
        orders[0] = [3, 4, 5, 6, 7, 0, 1, 2, 8]
        orders[-1] = [6, 7, 8, 0, 1, 2, 3, 4, 5]
        for b in range(nblocks):
            rb = blocks[b]
            r0 = sum(blocks[:b])  # first output row of this block in each half
            last = b == nblocks - 1
            ps_tiles = [
                psum_pool.tile([128, 4, 128], F32, name=f"ps_{b}_{j}", tag="ps")
                for j in range(rb * W // 512)
            ]
            for ti, t in enumerate(orders[b]):
                dh, dw = t // K - 1, t % K - 1
                dk_t = dk_pool.tile([128, rb, W], F32, name="dk_t", tag="dk")
                dk_src = AP(
                    dk_ap.tensor,
                    t * H * W + r0 * W,
                    [[NTAP * H * W, C], [HALF * W, 2], [W, rb], [1, W]],
                )
                nc.sync.dma_start(out=dk_t[:], in_=dk_src)
                # interleave the next x chunk a few taps into this block so
                # it lands well before block b+1's first mul
                if ti == 3 and b + 1 < nblocks and xdone < xcuts[b + 1]:
                    load_x_rows(xdone, xcuts[b + 1])
                    xdone = xcuts[b + 1]
                tmp = tmp_pool.tile([128, rb, W], BF16, name="tmp", tag="tmp")
                # valid output columns for this tap; the one boundary column
                # (reading x col -1 or W) contributes zero
                wo = slice(1, W) if dw < 0 else slice(0, W - 1) if dw > 0 else slice(0, W)
                wx = slice(wo.start + dw, wo.stop + dw)
                if dw != 0:
                    zc = slice(0, 1) if dw < 0 else slice(W - 1, W)
                    nc.gpsimd.memset(tmp[:, :, zc], 0.0)
                # rows whose x source row r0+r+dh is inside this half
                er = None  # (tmp row, halo row) needing the halo source
                rlo, rhi = 0, rb
                if b == 0 and dh < 0:
                    rlo, er = 1, (0, 1)  # top edge row: x row -1|63 = halo_top
                elif last and dh > 0:
                    rhi, er = rb - 1, (rb - 1, 0)  # bottom: x row 64|128 = halo_bot
                stop = ti == NTAP - 1
                if last and stop:
                    # closing tap of the final block: split the mul per psum
                    # tile so drain pipelines mul_j -> matmul_j -> copy_j
                    for j in range(len(ps_tiles)):
                        nc.vector.tensor_mul(
                            tmp[:, 4 * j : 4 * j + 4, wo],
                            x_tile[:, r0 + dh + 4 * j : r0 + dh + 4 * j + 4, wx],
                            dk_t[:, 4 * j : 4 * j + 4, wo],
                        )
                        nc.tensor.matmul(
                            ps_tiles[j][:],
                            identity[:],
                            tmp[:, 4 * j : 4 * j + 4, :],
                            start=False,
                            stop=True,
                        )
                    continue
                nc.vector.tensor_mul(
                    tmp[:, rlo:rhi, wo],
                    x_tile[:, r0 + dh + rlo : r0 + dh + rhi, wx],
                    dk_t[:, rlo:rhi, wo],
                )
                if er is not None:
                    tr, hr = er
                    nc.gpsimd.tensor_mul(
                        tmp[:, tr : tr + 1, wo],
                        halo[:, hr : hr + 1, wx],
                        dk_t[:, tr : tr + 1, wo],
                    )
                for j in range(len(ps_tiles)):
                    nc.tensor.matmul(
                        ps_tiles[j][:],
                        identity[:],
                        tmp[:, 4 * j : 4 * j + 4, :],
                        start=(ti == 0),
                        stop=stop,
                    )

            out_sb = out_pool.tile([128, rb, W], BF16, name="out_sb", tag="osb")
            for j in range(len(ps_tiles)):
                nc.scalar.copy(out=out_sb[:, 4 * j : 4 * j + 4, :], in_=ps_tiles[j][:])
                if last:
                    # per-tile store on the ACT ring right behind its copy:
                    # shortest possible drain tail
                    nc.scalar.dma_start(
                        out=AP(
                            out_ap.tensor,
                            (r0 + 4 * j) * W,
                            [[H * W, C], [HALF * W, 2], [W, 4], [1, W]],
                        ),
                        in_=out_sb[:, 4 * j : 4 * j + 4, :],
                    )
            if not last:
                out_dst = AP(
                    out_ap.tensor,
                    r0 * W,
                    [[H * W, C], [HALF * W, 2], [W, rb], [1, W]],
                )
                nc.gpsimd.dma_start(out=out_dst, in_=out_sb[:])
    finally:
        for p in reversed(ctx_pools):
            p.__exit__(None, None, None)


def build_nc():
    global _CACHED_NC
    if _CACHED_NC is not None:
        return _CACHED_NC
    nc = bacc.Bacc("TRN2", target_bir_lowering=False, debug=False, num_devices=NCORES)
    x_ap = nc.dram_tensor("x", [C, H, W], F32, kind="ExternalInput").ap()
    dk_ap = nc.dram_tensor(
        "dynamic_kernel", [C, NTAP, H, W], F32, kind="ExternalInput"
    ).ap()
    pm_dram = nc.dram_tensor("pmask", [128, 2], F32, kind="ExternalInput").ap()
    out_ap = nc.dram_tensor("out", [C, H, W], BF16, kind="ExternalOutput").ap()
    with tile.TileContext(nc) as tc:
        _emit(tc, nc, x_ap, dk_ap, pm_dram, out_ap)
    nc.compile()
    _CACHED_NC = nc
    return nc


def pmask_np() -> np.ndarray:
    p = np.arange(128)
    return np.stack([(p % 2 == 0), (p % 2 == 1)], axis=1).astype(np.float32)


def make_in_maps(x: np.ndarray, dynamic_kernel: np.ndarray, n: int = NCORES):
    pm = pmask_np()
    return [
        {
            "x": np.ascontiguousarray(x[i], dtype=np.float32),
            "dynamic_kernel": np.ascontiguousarray(dynamic_kernel[i], dtype=np.float32),
            "pmask": pm,
        }
        for i in range(n)
    ]


def kernel(x: np.ndarray, dynamic_kernel: np.ndarray) -> np.ndarray:
    x = np.asarray(x)
    dynamic_kernel = np.asarray(dynamic_kernel)
    nc = build_nc()
    in_maps = make_in_maps(x, dynamic_kernel)
    res = run_bass_kernel_spmd(nc, in_maps, core_ids=list(range(NCORES)))
    out = np.stack([res.results[i]["out"] for i in range(NCORES)], axis=0)
    return out.astype(np.float32)



# revision 6
# speedup vs baseline: 1.1448x; 1.1448x over previous
"""AdaConv2d (per-pixel 3x3 dynamic conv) on 8 TRN2 NeuronCores.

out[b,c,h,w] = sum_t x_pad[b,c,h+dh(t),w+dw(t)] * dk[b,c,t,h,w]

Sharding: pure data parallel over batch (B=8 -> one batch element per core).

Per-core layout: partition p = 2c+s (c = channel 0..63, s = H-half 0..1); each
partition holds its unpadded half-plane of x ([64 rows x 128 cols], row f =
x[c, 64s+f]) resident in SBUF while dynamic_kernel streams through in
(row-block, tap) tiles. Tap shifts are free-dim window reads; W-boundary
columns are zeroed in the product tile; H-boundary rows use parity-masked
halo row tiles (the mid-boundary halo lives in the neighbor partition, so it
is DMA'd separately and masked by a per-partition parity vector).

Engines: DVE computes the per-tap products (f32 in, bf16 out); the 9-tap sum
is accumulated on the TensorEngine via identity-matmul into PSUM (f32); ACT
drains PSUM to bf16 SBUF staging; output is stored bf16 and upcast on host.
Memory-bound: ~44 MB/core HBM traffic at ~360-400 GB/s/core.
"""

import numpy as np

from concourse import bacc, bass, tile
from concourse import mybir
from concourse.ap import AP
from concourse.bass_utils import run_bass_kernel_spmd
from concourse.masks import make_identity

F32 = mybir.dt.float32
BF16 = mybir.dt.bfloat16

B, C, H, W = 8, 64, 128, 128
K = 3
NTAP = K * K
NCORES = 8

HALF = H // 2           # 64 rows per half-plane
NBLK = 4                # row-blocks per half
RB = HALF // NBLK       # 16 rows per block
XROWS = HALF + 2        # 66 padded rows per partition
XCOLS = W + 2           # 130 padded cols

_CACHED_NC = None


def _emit(tc, nc, x_ap, dk_ap, pm_dram, out_ap):
    ctx_pools = []

    def pool(name, bufs, space=bass.MemorySpace.SBUF):
        p = tc.tile_pool(name=name, bufs=bufs, space=space)
        ctx_pools.append(p)
        return p.__enter__()

    try:
        const_pool = pool("const", 1)
        x_pool = pool("xp", 1)
        dk_pool = pool("dk", 6)
        tmp_pool = pool("tmp", 4)
        out_pool = pool("osb", 3)
        psum_pool = pool("ps", 8, space=bass.MemorySpace.PSUM)

        identity = const_pool.tile([128, 128], BF16, name="identity")
        make_identity(nc, identity)

        # Partition p = 2c+s (c = channel, s = H-half). x_tile row 1+f holds
        # x[c, 64s+f]; row 0 is the top halo (x row 63 of the other half,
        # nonzero only for odd p) and row 65 the bottom halo (x row 64,
        # nonzero only for even p). With halos folded into x_tile every
        # tap's product is ONE uniform full-height DVE mul — no edge-row
        # special cases on any critical path. W-boundary tap columns are
        # memset to zero in the product tile.
        pm_ap = const_pool.tile([128, 2], F32, name="pm_ap")
        nc.scalar.dma_start(out=pm_ap[:], in_=pm_dram)
        # halo_raw row 0: x row 64+s (s=0 -> row 64 = bottom halo, s=1 junk)
        # halo_raw row 1: x row 62+s (s=1 -> row 63 = top halo, s=0 junk)
        halo_raw = x_pool.tile([128, 2, W], F32, name="halo_raw")
        nc.scalar.dma_start(
            out=halo_raw[:, 0:1, :],
            in_=AP(x_ap.tensor, HALF * W, [[H * W, C], [W, 2], [1, W]]),
        )
        nc.scalar.dma_start(
            out=halo_raw[:, 1:2, :],
            in_=AP(x_ap.tensor, (HALF - 2) * W, [[H * W, C], [W, 2], [1, W]]),
        )
        # x (and its halo rows) live in one 66-row tile; bulk rows stream in
        # per-block just-in-time chunks on the SAME sync ring as dk — one
        # FIFO ring means block0's x rows land ~2us in, the DVE starts
        # immediately, and the dk stream never stalls on bufs backpressure
        # waiting for x (a separate-ring x load fair-shares the DMA engines
        # and finishes only at ~30us, idling everything).
        x_tile = x_pool.tile([128, HALF + 2, W], F32, name="x_tile")
        nc.gpsimd.tensor_scalar_mul(
            x_tile[:, 0:1, :], halo_raw[:, 1:2, :], pm_ap[:, 1:2]
        )
        nc.gpsimd.tensor_scalar_mul(
            x_tile[:, HALF + 1 : HALF + 2, :], halo_raw[:, 0:1, :], pm_ap[:, 0:1]
        )

        def load_x_rows(lo, hi):
            nc.sync.dma_start(
                out=x_tile[:, lo + 1 : hi + 1, :],
                in_=AP(
                    x_ap.tensor,
                    lo * W,
                    [[H * W, C], [HALF * W, 2], [W, hi - lo], [1, W]],
                ),
            )

        blocks = [16, 32, 16]  # >=8KB/lane packets for every dk load
        assert sum(blocks) == HALF
        nblocks = len(blocks)
        # x chunk b covers what block b reads (rows r0-1 .. r0+rb)
        xcuts = [18, 50, HALF]
        load_x_rows(0, xcuts[0])
        xdone = xcuts[0]
        # Tap order: block0 defers its dh=-1 taps (they read halo row 0,
        # masked on gpsimd ~13us in) so the DVE starts on plain taps; the
        # final block closes on the center tap (dw=0: no memset, cheapest
        # possible drain chain).
        orders = [list(range(NTAP))] * nblocks
        orders[0] = [3, 4, 5, 6, 7, 0, 1, 2, 8]
        orders[-1] = [0, 1, 2, 3, 5, 6, 7, 8, 4]
        for b in range(nblocks):
            rb = blocks[b]
            r0 = sum(blocks[:b])  # first output row of this block in each half
            last = b == nblocks - 1
            ps_tiles = [
                psum_pool.tile([128, 4, 128], F32, name=f"ps_{b}_{j}", tag="ps")
                for j in range(rb * W // 512)
            ]
            for ti, t in enumerate(orders[b]):
                dh, dw = t // K - 1, t % K - 1
                dk_t = dk_pool.tile([128, rb, W], F32, name="dk_t", tag="dk")
                dk_src = AP(
                    dk_ap.tensor,
                    t * H * W + r0 * W,
                    [[NTAP * H * W, C], [HALF * W, 2], [W, rb], [1, W]],
                )
                nc.sync.dma_start(out=dk_t[:], in_=dk_src)
                # interleave the next x chunk a few taps into this block so
                # it lands well before block b+1's first mul
                if ti == 3 and b + 1 < nblocks and xdone < xcuts[b + 1]:
                    load_x_rows(xdone, xcuts[b + 1])
                    xdone = xcuts[b + 1]
                tmp = tmp_pool.tile([128, rb, W], BF16, name="tmp", tag="tmp")
                # valid output columns for this tap; the one boundary column
                # (reading x col -1 or W) contributes zero
                wo = slice(1, W) if dw < 0 else slice(0, W - 1) if dw > 0 else slice(0, W)
                wx = slice(wo.start + dw, wo.stop + dw)
                if dw != 0:
                    zc = slice(0, 1) if dw < 0 else slice(W - 1, W)
                    nc.gpsimd.memset(tmp[:, :, zc], 0.0)
                xr = r0 + dh + 1  # first x_tile row this tap reads
                stop = ti == NTAP - 1
                if last and stop:
                    # closing tap of the final block: split the mul per psum
                    # tile so the drain pipelines mul_j -> matmul_j -> copy_j
                    for j in range(len(ps_tiles)):
                        nc.vector.tensor_mul(
                            tmp[:, 4 * j : 4 * j + 4, wo],
                            x_tile[:, xr + 4 * j : xr + 4 * j + 4, wx],
                            dk_t[:, 4 * j : 4 * j + 4, wo],
                        )
                        nc.tensor.matmul(
                            ps_tiles[j][:],
                            identity[:],
                            tmp[:, 4 * j : 4 * j + 4, :],
                            start=False,
                            stop=True,
                        )
                    continue
                nc.vector.tensor_mul(
                    tmp[:, :, wo],
                    x_tile[:, xr : xr + rb, wx],
                    dk_t[:, :, wo],
                )
                for j in range(len(ps_tiles)):
                    nc.tensor.matmul(
                        ps_tiles[j][:],
                        identity[:],
                        tmp[:, 4 * j : 4 * j + 4, :],
                        start=(ti == 0),
                        stop=stop,
                    )

            out_sb = out_pool.tile([128, rb, W], BF16, name="out_sb", tag="osb")
            for j in range(len(ps_tiles)):
                nc.scalar.copy(out=out_sb[:, 4 * j : 4 * j + 4, :], in_=ps_tiles[j][:])
            if last:
                # two half-block stores on the (idle by now) sync ring so the
                # first half streams out while the second half still drains
                for h in range(2):
                    hr = rb // 2
                    nc.sync.dma_start(
                        out=AP(
                            out_ap.tensor,
                            (r0 + h * hr) * W,
                            [[H * W, C], [HALF * W, 2], [W, hr], [1, W]],
                        ),
                        in_=out_sb[:, h * hr : (h + 1) * hr, :],
                    )
            else:
                out_dst = AP(
                    out_ap.tensor,
                    r0 * W,
                    [[H * W, C], [HALF * W, 2], [W, rb], [1, W]],
                )
                nc.gpsimd.dma_start(out=out_dst, in_=out_sb[:])
    finally:
        for p in reversed(ctx_pools):
            p.__exit__(None, None, None)


def build_nc():
    global _CACHED_NC
    if _CACHED_NC is not None:
        return _CACHED_NC
    nc = bacc.Bacc("TRN2", target_bir_lowering=False, debug=False, num_devices=NCORES)
    x_ap = nc.dram_tensor("x", [C, H, W], F32, kind="ExternalInput").ap()
    dk_ap = nc.dram_tensor(
        "dynamic_kernel", [C, NTAP, H, W], F32, kind="ExternalInput"
    ).ap()
    pm_dram = nc.dram_tensor("pmask", [128, 2], F32, kind="ExternalInput").ap()
    out_ap = nc.dram_tensor("out", [C, H, W], BF16, kind="ExternalOutput").ap()
    with tile.TileContext(nc) as tc:
        _emit(tc, nc, x_ap, dk_ap, pm_dram, out_ap)
    nc.compile()
    _CACHED_NC = nc
    return nc


def pmask_np() -> np.ndarray:
    p = np.arange(128)
    return np.stack([(p % 2 == 0), (p % 2 == 1)], axis=1).astype(np.float32)


def make_in_maps(x: np.ndarray, dynamic_kernel: np.ndarray, n: int = NCORES):
    pm = pmask_np()
    return [
        {
            "x": np.ascontiguousarray(x[i], dtype=np.float32),
            "dynamic_kernel": np.ascontiguousarray(dynamic_kernel[i], dtype=np.float32),
            "pmask": pm,
        }
        for i in range(n)
    ]


def kernel(x: np.ndarray, dynamic_kernel: np.ndarray) -> np.ndarray:
    x = np.asarray(x)
    dynamic_kernel = np.asarray(dynamic_kernel)
    nc = build_nc()
    in_maps = make_in_maps(x, dynamic_kernel)
    res = run_bass_kernel_spmd(nc, in_maps, core_ids=list(range(NCORES)))
    out = np.stack([res.results[i]["out"] for i in range(NCORES)], axis=0)
    return out.astype(np.float32)



# revision 7
# speedup vs baseline: 1.1729x; 1.0246x over previous
"""AdaConv2d (per-pixel 3x3 dynamic conv) on 8 TRN2 NeuronCores.

out[b,c,h,w] = sum_t x_pad[b,c,h+dh(t),w+dw(t)] * dk[b,c,t,h,w]

Sharding: pure data parallel over batch (B=8 -> one batch element per core).

Per-core layout: partition p = 2c+s (c = channel 0..63, s = H-half 0..1); each
partition holds its unpadded half-plane of x ([64 rows x 128 cols], row f =
x[c, 64s+f]) resident in SBUF while dynamic_kernel streams through in
(row-block, tap) tiles. Tap shifts are free-dim window reads; W-boundary
columns are zeroed in the product tile; H-boundary rows use parity-masked
halo row tiles (the mid-boundary halo lives in the neighbor partition, so it
is DMA'd separately and masked by a per-partition parity vector).

Engines: DVE computes the per-tap products (f32 in, bf16 out); the 9-tap sum
is accumulated on the TensorEngine via identity-matmul into PSUM (f32); ACT
drains PSUM to bf16 SBUF staging; output is stored bf16 and upcast on host.
Memory-bound: ~44 MB/core HBM traffic at ~360-400 GB/s/core.
"""

import numpy as np

from concourse import bacc, bass, tile
from concourse import mybir
from concourse.ap import AP
from concourse.bass_utils import run_bass_kernel_spmd
from concourse.masks import make_identity

F32 = mybir.dt.float32
BF16 = mybir.dt.bfloat16

B, C, H, W = 8, 64, 128, 128
K = 3
NTAP = K * K
NCORES = 8

HALF = H // 2           # 64 rows per half-plane
NBLK = 4                # row-blocks per half
RB = HALF // NBLK       # 16 rows per block
XROWS = HALF + 2        # 66 padded rows per partition
XCOLS = W + 2           # 130 padded cols

_CACHED_NC = None


def _emit(tc, nc, x_ap, dk_ap, pm_dram, out_ap):
    ctx_pools = []

    def pool(name, bufs, space=bass.MemorySpace.SBUF):
        p = tc.tile_pool(name=name, bufs=bufs, space=space)
        ctx_pools.append(p)
        return p.__enter__()

    try:
        const_pool = pool("const", 1)
        x_pool = pool("xp", 1)
        dk_pool = pool("dk", 5)
        tmp_pool = pool("tmp", 6)
        out_pool = pool("osb", 3)
        psum_pool = pool("ps", 8, space=bass.MemorySpace.PSUM)

        identity = const_pool.tile([128, 128], BF16, name="identity")
        make_identity(nc, identity)

        # Partition p = 2c+s (c = channel, s = H-half). x_tile row 1+f holds
        # x[c, 64s+f]; row 0 is the top halo (x row 63 of the other half,
        # nonzero only for odd p) and row 65 the bottom halo (x row 64,
        # nonzero only for even p): every tap's product is ONE uniform
        # full-height DVE mul. W-boundary tap columns are memset in tmp.
        pm_ap = const_pool.tile([128, 2], F32, name="pm_ap")
        nc.scalar.dma_start(out=pm_ap[:], in_=pm_dram)
        halo_raw = x_pool.tile([128, 2, W], F32, name="halo_raw")
        nc.scalar.dma_start(
            out=halo_raw[:, 0:1, :],
            in_=AP(x_ap.tensor, HALF * W, [[H * W, C], [W, 2], [1, W]]),
        )
        nc.scalar.dma_start(
            out=halo_raw[:, 1:2, :],
            in_=AP(x_ap.tensor, (HALF - 2) * W, [[H * W, C], [W, 2], [1, W]]),
        )
        x_tile = x_pool.tile([128, HALF + 2, W], F32, name="x_tile")
        nc.gpsimd.tensor_scalar_mul(
            x_tile[:, 0:1, :], halo_raw[:, 1:2, :], pm_ap[:, 1:2]
        )
        nc.gpsimd.tensor_scalar_mul(
            x_tile[:, HALF + 1 : HALF + 2, :], halo_raw[:, 0:1, :], pm_ap[:, 0:1]
        )

        def load_x_rows(lo, hi):
            nc.sync.dma_start(
                out=x_tile[:, lo + 1 : hi + 1, :],
                in_=AP(
                    x_ap.tensor,
                    lo * W,
                    [[H * W, C], [HALF * W, 2], [W, hi - lo], [1, W]],
                ),
            )

        def load_dk(t, rlo, rhi):
            dk_t = dk_pool.tile([128, rhi - rlo, W], F32, name="dk_t", tag="dk")
            nc.sync.dma_start(
                out=dk_t[:],
                in_=AP(
                    dk_ap.tensor,
                    t * H * W + rlo * W,
                    [[NTAP * H * W, C], [HALF * W, 2], [W, rhi - rlo], [1, W]],
                ),
            )
            return dk_t

        def col_slices(dw):
            # valid output columns for this tap; the one boundary column
            # (reading x col -1 or W) contributes zero
            wo = slice(1, W) if dw < 0 else slice(0, W - 1) if dw > 0 else slice(0, W)
            return wo, slice(wo.start + dw, wo.stop + dw)

        # ---- block 0: output rows 0..32, nine monolithic 32-row taps
        # (16KB/lane packets). Its drain happens mid-stream with slack.
        load_x_rows(0, 34)
        ps0 = [
            psum_pool.tile([128, 4, 128], F32, name=f"ps0_{j}", tag="ps")
            for j in range(8)
        ]
        order0 = [3, 4, 5, 6, 7, 0, 1, 2, 8]
        for ti, t in enumerate(order0):
            dh, dw = t // K - 1, t % K - 1
            dk_t = load_dk(t, 0, 32)
            if ti == 3:
                load_x_rows(34, HALF)
            tmp = tmp_pool.tile([128, 32, W], BF16, name="tmp", tag="tmp")
            wo, wx = col_slices(dw)
            if dw != 0:
                zc = slice(0, 1) if dw < 0 else slice(W - 1, W)
                nc.gpsimd.memset(tmp[:, :, zc], 0.0)
            nc.vector.tensor_mul(
                tmp[:, :, wo], x_tile[:, dh + 1 : dh + 33, wx], dk_t[:, :, wo]
            )
            for j in range(8):
                nc.tensor.matmul(
                    ps0[j][:],
                    identity[:],
                    tmp[:, 4 * j : 4 * j + 4, :],
                    start=(ti == 0),
                    stop=(ti == 8),
                )
        out_sb0 = out_pool.tile([128, 32, W], BF16, name="out_sb0", tag="osb")
        for j in range(8):
            nc.scalar.copy(out=out_sb0[:, 4 * j : 4 * j + 4, :], in_=ps0[j][:])
        nc.scalar.dma_start(
            out=AP(out_ap.tensor, 0, [[H * W, C], [HALF * W, 2], [W, 32], [1, W]]),
            in_=out_sb0[:],
        )

        # ---- group: output rows 32..64 as three sub-blocks (16/8/8 rows)
        # sharing mostly-32-row dk loads; the closing tap (t=4, center: no
        # memset) is loaded in three row-slices so the post-stream drain is
        # a single 8-row mul -> 2 matmuls -> copies -> store.
        G = 32
        sbs = []  # (group-row lo, hi, psum tiles)
        for lo, hi in [(0, 16), (16, 24), (24, 32)]:
            tiles = [
                psum_pool.tile([128, 4, 128], F32, name=f"psg_{lo}_{j}", tag="ps")
                for j in range((hi - lo) // 4)
            ]
            sbs.append((lo, hi, tiles))
        seq = [(t, 0, 32) for t in (5, 6, 7, 8, 0, 1, 2)]
        seq += [(3, 0, 16), (3, 16, 32), (4, 0, 16), (4, 16, 24), (4, 24, 32)]
        for t, glo, ghi in seq:
            dh, dw = t // K - 1, t % K - 1
            dk_t = load_dk(t, G + glo, G + ghi)
            wo, wx = col_slices(dw)
            for lo, hi, tiles in sbs:
                a, b_ = max(lo, glo), min(hi, ghi)
                if a >= b_:
                    continue
                tmp = tmp_pool.tile([128, b_ - a, W], BF16, name="tmp", tag="tmp")
                if dw != 0:
                    zc = slice(0, 1) if dw < 0 else slice(W - 1, W)
                    nc.gpsimd.memset(tmp[:, :, zc], 0.0)
                nc.vector.tensor_mul(
                    tmp[:, :, wo],
                    x_tile[:, G + a + dh + 1 : G + b_ + dh + 1, wx],
                    dk_t[:, a - glo : b_ - glo, wo],
                )
                for j in range(len(tiles)):
                    tr = lo + 4 * j  # group row of this psum tile
                    if tr < a or tr >= b_:
                        continue
                    nc.tensor.matmul(
                        tiles[j][:],
                        identity[:],
                        tmp[:, tr - a : tr - a + 4, :],
                        start=(t == 5),
                        stop=(t == 4),
                    )
                if t == 4:
                    # this sub-block just closed: drain + store immediately
                    out_sb = out_pool.tile(
                        [128, hi - lo, W], BF16, name=f"out_g{lo}", tag="osb"
                    )
                    for j in range(len(tiles)):
                        nc.scalar.copy(
                            out=out_sb[:, 4 * j : 4 * j + 4, :], in_=tiles[j][:]
                        )
                    nc.scalar.dma_start(
                        out=AP(
                            out_ap.tensor,
                            (G + lo) * W,
                            [[H * W, C], [HALF * W, 2], [W, hi - lo], [1, W]],
                        ),
                        in_=out_sb[:],
                    )
    finally:
        for p in reversed(ctx_pools):
            p.__exit__(None, None, None)


def build_nc():
    global _CACHED_NC
    if _CACHED_NC is not None:
        return _CACHED_NC
    nc = bacc.Bacc("TRN2", target_bir_lowering=False, debug=False, num_devices=NCORES)
    x_ap = nc.dram_tensor("x", [C, H, W], F32, kind="ExternalInput").ap()
    dk_ap = nc.dram_tensor(
        "dynamic_kernel", [C, NTAP, H, W], F32, kind="ExternalInput"
    ).ap()
    pm_dram = nc.dram_tensor("pmask", [128, 2], F32, kind="ExternalInput").ap()
    out_ap = nc.dram_tensor("out", [C, H, W], BF16, kind="ExternalOutput").ap()
    with tile.TileContext(nc) as tc:
        _emit(tc, nc, x_ap, dk_ap, pm_dram, out_ap)
    nc.compile()
    _CACHED_NC = nc
    return nc


def pmask_np() -> np.ndarray:
    p = np.arange(128)
    return np.stack([(p % 2 == 0), (p % 2 == 1)], axis=1).astype(np.float32)


def make_in_maps(x: np.ndarray, dynamic_kernel: np.ndarray, n: int = NCORES):
    pm = pmask_np()
    return [
        {
            "x": np.ascontiguousarray(x[i], dtype=np.float32),
            "dynamic_kernel": np.ascontiguousarray(dynamic_kernel[i], dtype=np.float32),
            "pmask": pm,
        }
        for i in range(n)
    ]


def kernel(x: np.ndarray, dynamic_kernel: np.ndarray) -> np.ndarray:
    x = np.asarray(x)
    dynamic_kernel = np.asarray(dynamic_kernel)
    nc = build_nc()
    in_maps = make_in_maps(x, dynamic_kernel)
    res = run_bass_kernel_spmd(nc, in_maps, core_ids=list(range(NCORES)))
    out = np.stack([res.results[i]["out"] for i in range(NCORES)], axis=0)
    return out.astype(np.float32)

